# revision 1
# baseline (speedup 1.0000x reference)
"""Trainium2 Bass kernel for nn_ContrastGFN (dense transformer w/ Hydra linear attention).

Contract: kernel(**inputs) takes the FULL unsharded inputs from setup_inputs()
and returns the FULL (4, 4096, 512) float32 output.

Sharding: 8 cores, each handles 2048 tokens (half of one batch; cores 2b and
2b+1 split batch b). The only cross-core dependency is the Hydra reduction
kvsum[b,h,:] = sum_s k_hat*v, exchanged with a pairwise (2-core) AllReduce of
16KB per core.

Host-side preprocessing (pure weight algebra, done once per call in numpy):
  - mix layer and lin1 are two consecutive linear layers -> folded:
      x1pre = x @ (W_mix[:E]@W1) + mix @ (W_mix[E:]@W1) + (b_mix@W1 + b1)
  - LN gains fold into the following matmul: g1->W2, g2->W3
  - per-head output proj folds into the combine proj: Wc[h] = W_ho[h] @ W_o[h]
  - all biases become per-feature columns added during PSUM eviction

On-device layout: activations are feature-major [feature->partition,
token->free] so every matmul chains without transposes (weights stationary,
activations moving). k/v are computed token-major (per-token L2 norms via
bn_stats; norm scale fused into the ACT PSUM-eviction). Matmuls in bf16,
PSUM accumulation + LN statistics in f32.
"""
import sys

sys.path.insert(0, '/opt/trn_rl_repo')

import numpy as np
import ml_dtypes

import concourse.bass as bass
import concourse.tile as tile
from concourse import bacc, mybir
from concourse.bass_utils import run_bass_kernel_spmd
from concourse.masks import make_identity

B, S, E, H, O, MIX = 4, 4096, 512, 8, 512, 512
P = 128
NCORES = 8
TOK = B * S // NCORES        # 2048 tokens per core
CH = 4                       # chunks per core
TN = TOK // CH               # 512 tokens per chunk
FT = E // P                  # 4 feature tiles of 128
TS = TN // P                 # 4 token sub-tiles per chunk
EPS = 1e-5

bf16 = mybir.dt.bfloat16
f32 = mybir.dt.float32
AF = mybir.ActivationFunctionType
ALU = mybir.AluOpType
nbf16 = ml_dtypes.bfloat16

_NC_CACHE = {}


def _ln_stats(nc, work, rows, ps_small, ones_col_bf, eps_col, ln_d, row0,
              act, sq_tag):
    """Per-token LN stats for feature-major bf16 act [P, FT, TN]: returns
    (rstd_bc, c_bc) broadcast tiles. DVE tile-sums + one ones-matmul per
    moment; row math on [1, TN]; partition-broadcast via DRAM bounce."""
    sq = work.tile([P, FT, TN], bf16, tag=sq_tag)
    for ft in range(FT):
        nc.scalar.activation(sq[:, ft], act[:, ft], AF.Square)
    t_acc = work.tile([P, TN], bf16, tag="lnacc")
    nc.vector.tensor_add(t_acc, act[:, 0], act[:, 1])
    nc.vector.tensor_add(t_acc, t_acc, act[:, 2])
    nc.vector.tensor_add(t_acc, t_acc, act[:, 3])
    q_acc = work.tile([P, TN], bf16, tag="lnacc2")
    nc.vector.tensor_add(q_acc, sq[:, 0], sq[:, 1])
    nc.vector.tensor_add(q_acc, q_acc, sq[:, 2])
    nc.vector.tensor_add(q_acc, q_acc, sq[:, 3])
    ps_s = ps_small.tile([1, TN], f32, tag="small")
    ps_q = ps_small.tile([1, TN], f32, tag="small")
    nc.tensor.matmul(ps_s, ones_col_bf, t_acc, start=True, stop=True)
    nc.tensor.matmul(ps_q, ones_col_bf, q_acc, start=True, stop=True)
    mu_r = rows.tile([1, TN], f32, tag="rowf")
    nc.vector.tensor_scalar_mul(mu_r, ps_s, 1.0 / E)
    var_r = rows.tile([1, TN], f32, tag="rowf")
    nc.vector.tensor_mul(var_r, mu_r, mu_r)
    nc.vector.scalar_tensor_tensor(out=var_r, in0=ps_q, scalar=1.0 / E,
                                   in1=var_r, op0=ALU.mult, op1=ALU.subtract)
    nc.scalar.activation(var_r, var_r, AF.Ln, bias=eps_col[0:1, :])
    nc.scalar.activation(var_r, var_r, AF.Exp, scale=-0.5)
    rstd_rb = rows.tile([1, TN], bf16, tag="rowbf")
    nc.scalar.activation(rstd_rb, var_r, AF.Copy)
    c_rb = rows.tile([1, TN], bf16, tag="rowbf")
    nc.vector.tensor_mul(c_rb, mu_r, var_r)
    nc.gpsimd.dma_start(out=ln_d[row0:row0 + 1, :], in_=rstd_rb)
    nc.gpsimd.dma_start(out=ln_d[row0 + 1:row0 + 2, :], in_=c_rb)
    rstd = work.tile([P, TN], bf16, tag="rstdbc")
    nc.gpsimd.dma_start(
        out=rstd, in_=bass.AP(tensor=ln_d.ap().tensor, offset=row0 * TN,
                              ap=[[0, P], [1, TN]]))
    cbc = work.tile([P, TN], bf16, tag="cbc")
    nc.gpsimd.dma_start(
        out=cbc, in_=bass.AP(tensor=ln_d.ap().tensor, offset=(row0 + 1) * TN,
                             ap=[[0, P], [1, TN]]))
    return rstd, cbc


def _ln_apply(nc, act, rstd, cbc):
    for ft in range(FT):
        nc.vector.tensor_mul(act[:, ft], act[:, ft], rstd)
        nc.vector.tensor_sub(act[:, ft], act[:, ft], cbc)


def _build(has_qkv_bias, has_mask):
    nc = bacc.Bacc("TRN2", num_devices=NCORES)
    assert nc.vector.BN_STATS_FMAX >= E

    dp = nc.declare_dram_parameter
    x_d = dp("x", [TOK, E], f32, isOutput=False)
    mixcol_d = dp("mixcol", [P, FT], bf16, isOutput=False)
    wmm1_d = dp("wmm1", [P, FT, E], bf16, isOutput=False)
    wfold_d = dp("wfold", [P, FT, E], bf16, isOutput=False)
    w2p_d = dp("w2p", [P, FT, E], bf16, isOutput=False)
    w3p_d = dp("w3p", [P, FT, O], bf16, isOutput=False)
    wq_d = dp("wq", [H, P, FT, E], bf16, isOutput=False)
    wk_d = dp("wk", [H, P, FT, E], bf16, isOutput=False)
    wv_d = dp("wv", [H, P, FT, E], bf16, isOutput=False)
    wc_d = dp("wc", [H, P, FT, O], bf16, isOutput=False)
    bfoldc_d = dp("bfoldc", [P, FT], f32, isOutput=False)
    b2pc_d = dp("b2pc", [P, FT], f32, isOutput=False)
    bcc_d = dp("bcc", [P, FT], f32, isOutput=False)
    b3pc_d = dp("b3pc", [P, FT], f32, isOutput=False)
    if has_qkv_bias:
        bqrow_d = dp("bqrow", [H, E], bf16, isOutput=False)
        bkrow_d = dp("bkrow", [H, E], bf16, isOutput=False)
        bvrow_d = dp("bvrow", [H, E], bf16, isOutput=False)
    if has_mask:
        maskcol_d = dp("maskcol", [P, TOK // P], f32, isOutput=False)
    out_d = dp("out", [TOK, E], f32, isOutput=True)

    # internal DRAM (min shape [8, 512] to satisfy NEFF load)
    cc_in_a = nc.dram_tensor("cc_in_a", [H, E], f32)
    cc_out_a = nc.dram_tensor("cc_out_a", [H, E], f32)
    cc_in_b = nc.dram_tensor("cc_in_b", [H, E], f32)
    cc_out_b = nc.dram_tensor("cc_out_b", [H, E], f32)
    rnq_d = nc.dram_tensor("rnq_d", [CH * H, TN], bf16)
    ln_d = nc.dram_tensor("ln_d", [4 * CH, TN], bf16)

    with tile.TileContext(nc) as tc:
        import contextlib
        ctx = contextlib.ExitStack()
        with ctx:
            singles = ctx.enter_context(tc.tile_pool(name="singles", bufs=1))
            work = ctx.enter_context(tc.tile_pool(name="work", bufs=2))
            wpool = ctx.enter_context(tc.tile_pool(name="wpool", bufs=2))
            rows = ctx.enter_context(tc.tile_pool(name="rows", bufs=4))
            qspool = ctx.enter_context(tc.tile_pool(name="qspool", bufs=5))
            qaux = ctx.enter_context(tc.tile_pool(name="qaux", bufs=4))
            ps_big = ctx.enter_context(
                tc.tile_pool(name="ps_big", bufs=6, space="PSUM"))
            ps_small = ctx.enter_context(
                tc.tile_pool(name="ps_small", bufs=2, space="PSUM"))

            # ---- constants / resident weights ----
            ident_f = singles.tile([P, P], f32)
            make_identity(nc, ident_f)
            ident_b = singles.tile([P, P], bf16)
            make_identity(nc, ident_b)
            ones_col_bf = singles.tile([P, 1], bf16)
            nc.vector.memset(ones_col_bf, 1.0)
            eps_col = singles.tile([P, 1], f32)
            nc.vector.memset(eps_col, EPS)
            if has_qkv_bias:
                ones_row_tn = singles.tile([1, TN], bf16)
                nc.vector.memset(ones_row_tn, 1.0)

            wfold_sb = singles.tile([P, FT, E], bf16)
            nc.sync.dma_start(out=wfold_sb, in_=wfold_d[:, :, :])
            w2p_sb = singles.tile([P, FT, E], bf16)
            nc.sync.dma_start(out=w2p_sb, in_=w2p_d[:, :, :])
            w3p_sb = singles.tile([P, FT, O], bf16)
            nc.sync.dma_start(out=w3p_sb, in_=w3p_d[:, :, :])
            wmm1_sb = singles.tile([P, FT, E], bf16)
            nc.sync.dma_start(out=wmm1_sb, in_=wmm1_d[:, :, :])
            mixcol = singles.tile([P, FT], bf16)
            nc.sync.dma_start(out=mixcol, in_=mixcol_d[:, :])
            bfoldc = singles.tile([P, FT], f32)
            nc.sync.dma_start(out=bfoldc, in_=bfoldc_d[:, :])
            b2pc = singles.tile([P, FT], f32)
            nc.sync.dma_start(out=b2pc, in_=b2pc_d[:, :])
            bcc = singles.tile([P, FT], f32)
            nc.sync.dma_start(out=bcc, in_=bcc_d[:, :])
            b3pc = singles.tile([P, FT], f32)
            nc.sync.dma_start(out=b3pc, in_=b3pc_d[:, :])
            if has_qkv_bias:
                bqrow = singles.tile([H, E], bf16)
                nc.sync.dma_start(out=bqrow, in_=bqrow_d[:, :])
                bkrow = singles.tile([H, E], bf16)
                nc.sync.dma_start(out=bkrow, in_=bkrow_d[:, :])
                bvrow = singles.tile([H, E], bf16)
                nc.sync.dma_start(out=bvrow, in_=bvrow_d[:, :])
            if has_mask:
                maskcol = singles.tile([P, TOK // P], f32)
                nc.sync.dma_start(out=maskcol, in_=maskcol_d[:, :])

            x2stash = singles.tile([P, FT, TOK], bf16)
            kvcols = singles.tile([P, H * FT], f32)

            # ---- mix matvec: mveccol = mix @ (Wmm@W1) + (b_mix@W1 + b1) ----
            mveccol = singles.tile([P, FT], f32)
            for fo in range(FT):
                ps_mv = ps_big.tile([P, 1], f32, tag="big")
                for fin in range(FT):
                    nc.tensor.matmul(ps_mv,
                                     wmm1_sb[:, fin, fo * P:(fo + 1) * P],
                                     mixcol[:, fin:fin + 1],
                                     start=(fin == 0), stop=(fin == FT - 1))
                nc.scalar.activation(mveccol[:, fo:fo + 1], ps_mv, AF.Identity,
                                     bias=bfoldc[:, fo:fo + 1])

            # ---- phase A (pipelined): x -> t=gelu(x@Wfold+mvec) -> LN1 -> x2 ----
            def a_produce(c):
                xnat = work.tile([P, TS, E], f32, tag="xnat", name=f"xnat{c}")
                nc.sync.dma_start(
                    out=xnat,
                    in_=x_d[c * TN:(c + 1) * TN, :].rearrange(
                        "(ts p) e -> p ts e", p=P))
                xT = work.tile([P, FT, TN], bf16, tag="xT", name=f"xT{c}")
                for ts in range(TS):
                    for fe in range(FT):
                        ps_t = ps_big.tile([P, P], f32, tag="big",
                                           name=f"pstA{c}_{ts}_{fe}")
                        nc.tensor.transpose(
                            ps_t, xnat[:, ts, fe * P:(fe + 1) * P], ident_f)
                        nc.scalar.activation(
                            xT[:, fe, ts * P:(ts + 1) * P], ps_t, AF.Copy)
                t = work.tile([P, FT, TN], bf16, tag="t", name=f"t{c}")
                for fo in range(FT):
                    ps_1 = ps_big.tile([P, TN], f32, tag="big",
                                       name=f"ps1_{c}_{fo}")
                    for fin in range(FT):
                        nc.tensor.matmul(ps_1,
                                         wfold_sb[:, fin, fo * P:(fo + 1) * P],
                                         xT[:, fin, :],
                                         start=(fin == 0), stop=(fin == FT - 1))
                    nc.scalar.activation(t[:, fo], ps_1, AF.Gelu,
                                         bias=mveccol[:, fo:fo + 1])
                return t

            def a_finish(c, t, rstd, cbc):
                _ln_apply(nc, t, rstd, cbc)
                for fo in range(FT):
                    ps_2 = ps_big.tile([P, TN], f32, tag="big",
                                       name=f"ps2_{c}_{fo}")
                    for fin in range(FT):
                        nc.tensor.matmul(ps_2,
                                         w2p_sb[:, fin, fo * P:(fo + 1) * P],
                                         t[:, fin, :],
                                         start=(fin == 0), stop=(fin == FT - 1))
                    nc.scalar.activation(
                        x2stash[:, fo, c * TN:(c + 1) * TN], ps_2, AF.Gelu,
                        bias=b2pc[:, fo:fo + 1])

            pendA = None
            for c in range(CH):
                t = a_produce(c)
                st = _ln_stats(nc, work, rows, ps_small, ones_col_bf, eps_col,
                               ln_d, 4 * c, t, "sq")
                if pendA is not None:
                    a_finish(*pendA)
                pendA = (c, t) + st

            # ---- phase B: per head k,v (token-major) -> kvsum ----
            # (a_finish(3) is emitted after head 0's chunks 0-2 so the PE can
            #  chew on k/v matmuls while LN1(3) drains on DVE/ACT/DMA)
            for h in range(H):
                wk_sb = wpool.tile([P, FT, E], bf16, tag="wa", name=f"wk{h}")
                nc.sync.dma_start(out=wk_sb, in_=wk_d[h])
                wv_sb = wpool.tile([P, FT, E], bf16, tag="wb", name=f"wv{h}")
                nc.sync.dma_start(out=wv_sb, in_=wv_d[h])
                ps_kvs = ps_small.tile([1, E], f32, tag="small",
                                       name=f"pskvs{h}")
                for c in range(CH):
                    ksb = work.tile([P, TS, E], bf16, tag="ksb",
                                    name=f"ksb{h}_{c}")
                    vsb = work.tile([P, TS, E], bf16, tag="vsb",
                                    name=f"vsb{h}_{c}")
                    s2p = work.tile([P, TS], f32, tag="s2p", name=f"s2p{h}_{c}")
                    for ts in range(TS):
                        t0 = c * TN + ts * P
                        psk = ps_big.tile([P, E], f32, tag="big",
                                          name=f"psk{h}_{c}_{ts}")
                        psv = ps_big.tile([P, E], f32, tag="big",
                                          name=f"psv{h}_{c}_{ts}")
                        nmm = FT + (1 if has_qkv_bias else 0)
                        i = 0
                        if has_qkv_bias:
                            nc.tensor.matmul(psk, ones_row_tn[:, 0:P],
                                             bkrow[h:h + 1, :],
                                             start=True, stop=False)
                            nc.tensor.matmul(psv, ones_row_tn[:, 0:P],
                                             bvrow[h:h + 1, :],
                                             start=True, stop=False)
                            i = 1
                        for fin in range(FT):
                            nc.tensor.matmul(psk, x2stash[:, fin, t0:t0 + P],
                                             wk_sb[:, fin, :],
                                             start=(i + fin == 0),
                                             stop=(i + fin == nmm - 1))
                        for fin in range(FT):
                            nc.tensor.matmul(psv, x2stash[:, fin, t0:t0 + P],
                                             wv_sb[:, fin, :],
                                             start=(i + fin == 0),
                                             stop=(i + fin == nmm - 1))
                        kst = work.tile([P, 6], f32, tag="kst")
                        nc.vector.bn_stats(out=kst, in_=psk)
                        kmv = work.tile([P, 2], f32, tag="kmv")
                        nc.vector.bn_aggr(out=kmv, in_=kst)
                        nc.vector.tensor_mul(s2p[:, ts:ts + 1], kmv[:, 0:1],
                                             kmv[:, 0:1])
                        nc.vector.tensor_add(s2p[:, ts:ts + 1],
                                             s2p[:, ts:ts + 1], kmv[:, 1:2])
                        nc.scalar.activation(ksb[:, ts], psk, AF.Copy)
                        nc.scalar.activation(vsb[:, ts], psv, AF.Copy)
                    rn4 = work.tile([P, TS], f32, tag="rn4", name=f"rn4{h}_{c}")
                    nc.scalar.activation(rn4, s2p, AF.Ln, scale=float(E))
                    nc.scalar.activation(rn4, rn4, AF.Exp, scale=-0.5)
                    if has_mask:
                        nc.vector.tensor_mul(
                            rn4, rn4, maskcol[:, c * TS:(c + 1) * TS])
                    kv_acc = work.tile([P, E], bf16, tag="kvacc",
                                       name=f"kvacc{h}_{c}")
                    nc.vector.scalar_tensor_tensor(
                        out=kv_acc, in0=ksb[:, 0], scalar=rn4[:, 0:1],
                        in1=vsb[:, 0], op0=ALU.mult, op1=ALU.mult)
                    for ts in range(1, TS):
                        kvt = work.tile([P, E], bf16, tag="kvt")
                        nc.vector.scalar_tensor_tensor(
                            out=kvt, in0=ksb[:, ts], scalar=rn4[:, ts:ts + 1],
                            in1=vsb[:, ts], op0=ALU.mult, op1=ALU.mult)
                        nc.vector.tensor_add(kv_acc, kv_acc, kvt)
                    nc.tensor.matmul(ps_kvs, ones_col_bf, kv_acc,
                                     start=(c == 0), stop=(c == CH - 1))
                    if h == 0 and c == CH - 2 and pendA is not None:
                        a_finish(*pendA)
                        pendA = None
                kvrow = rows.tile([1, E], f32, tag="rowf")
                nc.scalar.activation(kvrow, ps_kvs, AF.Copy)
                cc = cc_in_a if h < H // 2 else cc_in_b
                nc.gpsimd.dma_start(out=cc[h:h + 1, :], in_=kvrow)
                if h == H // 2 - 1:
                    # first-half AllReduce overlaps the back half of phase B
                    nc.gpsimd.collective_compute(
                        "AllReduce", ALU.add,
                        replica_groups=[[0, 1], [2, 3], [4, 5], [6, 7]],
                        ins=[cc_in_a[:]], outs=[cc_out_a[:]])
                    nc.gpsimd.dma_start(
                        out=kvcols[:, 0:H * FT // 2],
                        in_=cc_out_a.ap().rearrange(
                            "h (t p) -> p (h t)", p=P)[:, 0:H * FT // 2])

            # ---- second-half kvsum AllReduce ----
            nc.gpsimd.collective_compute(
                "AllReduce", ALU.add,
                replica_groups=[[0, 1], [2, 3], [4, 5], [6, 7]],
                ins=[cc_in_b[:]], outs=[cc_out_b[:]])
            nc.gpsimd.dma_start(
                out=kvcols[:, H * FT // 2:],
                in_=cc_out_b.ap().rearrange(
                    "h (t p) -> p (h t)", p=P)[:, H * FT // 2:])

            # ---- phase 2 + C (interleaved): q/attn, then LN2/W3/residual ----
            def qpart(c, h):
                wq_sb = wpool.tile([P, FT, E], bf16, tag="wa",
                                   name=f"wq{c}_{h}")
                nc.sync.dma_start(out=wq_sb, in_=wq_d[h])
                wc_sb = wpool.tile([P, FT, O], bf16, tag="wb",
                                   name=f"wc{c}_{h}")
                nc.sync.dma_start(out=wc_sb, in_=wc_d[h])
                wc_sb_cache[(c, h)] = wc_sb
                qs = qspool.tile([P, FT, TN], bf16, tag="qs",
                                 name=f"qs{c}_{h}")
                qsq_acc = qaux.tile([P, TN], bf16, tag="qsqa",
                                    name=f"qsqa{c}_{h}")
                for et in range(FT):
                    psq = ps_big.tile([P, TN], f32, tag="big",
                                      name=f"psq{c}_{h}_{et}")
                    i = 0
                    nmm = FT + (1 if has_qkv_bias else 0)
                    if has_qkv_bias:
                        nc.tensor.matmul(
                            psq, bqrow[h:h + 1, et * P:(et + 1) * P],
                            ones_row_tn, start=True, stop=False)
                        i = 1
                    for fin in range(FT):
                        nc.tensor.matmul(
                            psq, wq_sb[:, fin, et * P:(et + 1) * P],
                            x2stash[:, fin, c * TN:(c + 1) * TN],
                            start=(i + fin == 0), stop=(i + fin == nmm - 1))
                    if et == 0:
                        nc.scalar.activation(qsq_acc, psq, AF.Square)
                    else:
                        qsq = qaux.tile([P, TN], bf16, tag="qsq")
                        nc.scalar.activation(qsq, psq, AF.Square)
                        nc.vector.tensor_add(qsq_acc, qsq_acc, qsq)
                    nc.scalar.activation(qs[:, et], psq, AF.Copy)
                ps_ns = ps_small.tile([1, TN], f32, tag="small",
                                      name=f"psns{c}_{h}")
                nc.tensor.matmul(ps_ns, ones_col_bf, qsq_acc,
                                 start=True, stop=True)
                sdq = rows.tile([1, TN], f32, tag="rowf")
                nc.scalar.activation(sdq, ps_ns, AF.Ln)
                rnq_row = rows.tile([1, TN], bf16, tag="rnqrow")
                nc.scalar.activation(rnq_row, sdq, AF.Exp, scale=-0.5)
                r = c * H + h
                nc.gpsimd.dma_start(out=rnq_d[r:r + 1, :], in_=rnq_row)
                rnq_bc = qaux.tile([P, TN], bf16, tag="rnqbc")
                nc.gpsimd.dma_start(
                    out=rnq_bc,
                    in_=bass.AP(tensor=rnq_d.ap().tensor, offset=r * TN,
                                ap=[[0, P], [1, TN]]))
                return qs, rnq_bc

            def attnmm(c, h, at, qsr):
                qs, rnq_bc = qsr
                for et in range(FT):
                    nc.vector.scalar_tensor_tensor(
                        out=qs[:, et], in0=qs[:, et],
                        scalar=kvcols[:, h * FT + et:h * FT + et + 1],
                        in1=rnq_bc, op0=ALU.mult, op1=ALU.mult)
                for fo in range(FT):
                    for fin in range(FT):
                        nc.tensor.matmul(
                            at[fo], wc_sb_cache[(c, h)][:, fin,
                                                        fo * P:(fo + 1) * P],
                            qs[:, fin, :],
                            start=(h == 0 and fin == 0),
                            stop=(h == H - 1 and fin == FT - 1))

            def c_tail_pre(c, at):
                a_t = work.tile([P, FT, TN], bf16, tag="a_t", name=f"a_t{c}")
                for fo in range(FT):
                    nc.scalar.activation(a_t[:, fo], at[fo], AF.Identity,
                                         bias=bcc[:, fo:fo + 1])
                st = _ln_stats(nc, work, rows, ps_small, ones_col_bf, eps_col,
                               ln_d, 4 * c + 2, a_t, "sq")
                return (c, a_t) + st

            def c_tail_post(c, a_t, rstd, cbc):
                _ln_apply(nc, a_t, rstd, cbc)
                g3 = work.tile([P, FT, TN], bf16, tag="g3", name=f"g3{c}")
                for fo in range(FT):
                    ps_3 = ps_big.tile([P, TN], f32, tag="big",
                                       name=f"ps3_{c}_{fo}")
                    for fin in range(FT):
                        nc.tensor.matmul(ps_3,
                                         w3p_sb[:, fin, fo * P:(fo + 1) * P],
                                         a_t[:, fin, :],
                                         start=(fin == 0), stop=(fin == FT - 1))
                    nc.scalar.activation(g3[:, fo], ps_3, AF.Gelu,
                                         bias=b3pc[:, fo:fo + 1])
                xr = work.tile([P, TS, E], f32, tag="xnat", name=f"xr{c}")
                nc.sync.dma_start(
                    out=xr,
                    in_=x_d[c * TN:(c + 1) * TN, :].rearrange(
                        "(ts p) e -> p ts e", p=P))
                for ts in range(TS):
                    for fo in range(FT):
                        ps_t = ps_big.tile([P, P], bf16, tag="big",
                                           name=f"pstC{c}_{ts}_{fo}")
                        nc.tensor.transpose(
                            ps_t, g3[:, fo, ts * P:(ts + 1) * P], ident_b)
                        nc.vector.tensor_add(
                            xr[:, ts, fo * P:(fo + 1) * P], ps_t,
                            xr[:, ts, fo * P:(fo + 1) * P])
                nc.gpsimd.dma_start(
                    out=out_d[c * TN:(c + 1) * TN, :].rearrange(
                        "(ts p) e -> p ts e", p=P),
                    in_=xr)

            wc_sb_cache = {}
            pendC = None
            for c in range(CH):
                pre = 4 if c == 0 else 3
                qsd = {}
                for h in range(pre):
                    qsd[h] = qpart(c, h)
                if pendC is not None:
                    c_tail_post(*pendC)
                    pendC = None
                at = [ps_big.tile([P, TN], f32, tag="big", name=f"at{c}_{i}")
                      for i in range(FT)]
                for h in range(H):
                    if h + pre < H:
                        qsd[h + pre] = qpart(c, h + pre)
                    attnmm(c, h, at, qsd.pop(h))
                pendC = c_tail_pre(c, at)
            c_tail_post(*pendC)
    nc.compile()
    return nc


def _get_nc(has_qkv_bias, has_mask):
    key = (has_qkv_bias, has_mask)
    if key not in _NC_CACHE:
        _NC_CACHE[key] = _build(has_qkv_bias, has_mask)
    return _NC_CACHE[key]


def _wlayout(w):
    """[K, M] weight -> [P, K//P, M] stationary layout, bf16, contiguous."""
    k, m = w.shape
    return np.ascontiguousarray(
        w.reshape(k // P, P, m).transpose(1, 0, 2)).astype(nbf16)


def _col(v):
    """[E] per-feature vector -> [P, FT] column layout (f32)."""
    return np.ascontiguousarray(v.reshape(-1, P).T).astype(np.float32)


def _prep(x, mix, mask, W_mix, b_mix, W1, b1, g1, bt1, W2, b2,
          W_qkv, b_qkv, W_ho, b_ho, W_o, b_o, g2, bt2, W3, b3):
    f = np.float32
    x = np.asarray(x, f)
    mix = np.asarray(mix, f)
    mask = np.asarray(mask)
    W_mix = np.asarray(W_mix, f); b_mix = np.asarray(b_mix, f)
    W1 = np.asarray(W1, f); b1 = np.asarray(b1, f)
    g1 = np.asarray(g1, f); bt1 = np.asarray(bt1, f)
    W2 = np.asarray(W2, f); b2 = np.asarray(b2, f)
    W_qkv = np.asarray(W_qkv, f); b_qkv = np.asarray(b_qkv, f)
    W_ho = np.asarray(W_ho, f); b_ho = np.asarray(b_ho, f)
    W_o = np.asarray(W_o, f); b_o = np.asarray(b_o, f)
    g2 = np.asarray(g2, f); bt2 = np.asarray(bt2, f)
    W3 = np.asarray(W3, f); b3 = np.asarray(b3, f)

    wfold = W_mix[:E] @ W1                     # [E, E]
    wmm1 = W_mix[E:] @ W1                      # [MIX, E]
    bfold = b_mix @ W1 + b1                    # [E]
    w2p = (g1[:, None] * W2)                   # [E, E]
    b2p = bt1 @ W2 + b2
    wc = np.stack([W_ho[h] @ W_o[h * O:(h + 1) * O] for h in range(H)])
    bc = sum(b_ho[h] @ W_o[h * O:(h + 1) * O] for h in range(H)) + b_o
    w3p = (g2[:, None] * W3)
    b3p = bt2 @ W3 + b3
    wq = W_qkv[:, :, 0:E]
    wk = W_qkv[:, :, E:2 * E]
    wv = W_qkv[:, :, 2 * E:3 * E]
    bq = b_qkv[:, 0:E]
    bk = b_qkv[:, E:2 * E]
    bv = b_qkv[:, 2 * E:3 * E]

    has_qkv_bias = bool(np.any(b_qkv != 0))
    has_mask = bool(np.any(mask))

    shared = {
        "mixcol": None,  # per-core (depends on batch)
        "wmm1": _wlayout(wmm1),
        "wfold": _wlayout(wfold),
        "w2p": _wlayout(w2p),
        "w3p": _wlayout(w3p),
        "wq": np.stack([_wlayout(wq[h]) for h in range(H)]),
        "wk": np.stack([_wlayout(wk[h]) for h in range(H)]),
        "wv": np.stack([_wlayout(wv[h]) for h in range(H)]),
        "wc": np.stack([_wlayout(wc[h]) for h in range(H)]),
        "bfoldc": _col(bfold),
        "b2pc": _col(b2p),
        "bcc": _col(bc),
        "b3pc": _col(b3p),
    }
    in_maps = []
    for core in range(NCORES):
        b = core // 2
        s0 = (core % 2) * TOK
        m = {k: v for k, v in shared.items() if v is not None}
        m["x"] = np.ascontiguousarray(x[b, s0:s0 + TOK, :])
        m["mixcol"] = np.ascontiguousarray(
            mix[b].reshape(FT, P).T).astype(nbf16)
        if has_qkv_bias:
            m["bqrow"] = bq.astype(nbf16)
            m["bkrow"] = bk.astype(nbf16)
            m["bvrow"] = bv.astype(nbf16)
        if has_mask:
            mm = 1.0 - mask[b, s0:s0 + TOK].astype(np.float32)
            m["maskcol"] = np.ascontiguousarray(
                mm.reshape(TOK // P, P).T).astype(np.float32)
        in_maps.append(m)
    return in_maps, has_qkv_bias, has_mask


def _run(in_maps, has_qkv_bias, has_mask, **kw):
    nc = _get_nc(has_qkv_bias, has_mask)
    res = run_bass_kernel_spmd(nc, in_maps, list(range(NCORES)), **kw)
    out = np.empty((B, S, E), np.float32)
    for core in range(NCORES):
        b = core // 2
        s0 = (core % 2) * TOK
        out[b, s0:s0 + TOK, :] = res.results[core]["out"]
    return out, res


def kernel(**inputs):
    in_maps, hb, hm = _prep(**inputs)
    out, _ = _run(in_maps, hb, hm)
    return out


def kernel_profiled(tmpdir=None, **inputs):
    """Like kernel(), but also returns exec_time_ns from the NTFF profile."""
    in_maps, hb, hm = _prep(**inputs)
    out, res = _run(in_maps, hb, hm, trace=True, tmpdir=tmpdir)
    return out, res

